# revision 60
# baseline (speedup 1.0000x reference)
"""Trainium2 Bass kernel for nn_BiLSTM pairwise-scores problem.

Math (reference):
  vec  = concat(word_emb[wi], pos_emb[pi], ext_emb[ei])          [512, 425]
  h    = concat(lstm_cell_f(vec), lstm_cell_b(vec))              [512, 200]
  cat  = [h, vec] for t <= 255 else [vec, h]                     [512, 625]
  f    = cat @ w_mlp_in.T + b_mlp_in                             [512, 400]
  out  = tanh((f[:,None,:] + f[None,:,:]) @ w_mlp_out.T + b_out) [512, 512, 42]

Key factorizations:
 1. (f_i + f_j) @ Wout.T + b = g'_i + g'_j with g' = f @ Wout.T + b/2,
    so the O(n^2*400*42) matmul collapses to a [n, 42] projection plus a
    pairwise broadcast-add, implemented on the PE as a single K=43 matmul
    per output chunk: lhsT = [g'_iT rows; ones row], rhs = [periodic
    identity rows; g'_j flattened row].
 2. There is no nonlinearity between mlp_in and mlp_out, so
    g' = cat @ W2.T + b2 with W2 = Wout @ Wmlp_in  [42, 625] and
    b2 = Wout @ b_mlp_in + b_out/2, both precomputed on the host: the
    entire 400-wide mlp_in stage disappears from the device program.
    The b2 bias rides a ones-row baked into the vt3 k-tile (no DVE adds).

Sharding: 8 cores = 4 i-blocks (128 rows) x 2 j-halves (256 cols).
Each core runs an identical (SPMD) program on a permuted 384-token slice:
cols 0:128 = its i-block tokens, cols 128:384 = its j-half tokens.
The embedding gather and weight layout happen on the host; all dense
compute (LSTM cells, fused mlp, pairwise + tanh) runs on device in bf16
with fp32 PSUM accumulation. Output DMAs in bf16; host upcasts to f32.
"""

import os
import sys

import numpy as np

for _p in ("/opt/trn_rl_repo", "/root/.axon_site/_ro/trn_rl_repo"):
    if os.path.isdir(_p) and _p not in sys.path:
        sys.path.insert(0, _p)

import ml_dtypes  # noqa: E402

import concourse.bacc as bacc  # noqa: E402
import concourse.bass as bass  # noqa: E402
import concourse.mybir as mybir  # noqa: E402
from concourse.bass_utils import run_bass_kernel_spmd  # noqa: E402
from concourse.tile import TileContext  # noqa: E402

BF16 = mybir.dt.bfloat16
FP8 = mybir.dt.float8e4
F32 = mybir.dt.float32
AF = mybir.ActivationFunctionType
DOUBLE_ROW = mybir.MatmulPerfMode.DoubleRow

SEQ = 512
D_VEC = 425  # 100 + 25 + 300
NREL = 42
T = 384  # per-core tokens: 128 (i-block) + 256 (j-half)
NFLAT = 256 * NREL  # 10752 = per-core output row length
N_CHUNK = 512
N_CHUNKS = NFLAT // N_CHUNK  # 21
GRP = 3  # pairwise chunks fused per PSUM group / tanh / DMA
IC_PER = 32 * NREL  # 1344: replication period for the identity pattern

# K-dim tiling of the 425-dim feature axis. vt3 carries an extra all-ones
# row 41 so the fused-mlp bias b2 folds in as a rank-1 matmul term.
KS = [(0, 128), (128, 256), (256, 384), (384, 425)]
VT_ROWS = [128, 128, 128, 42]  # vt3 = 41 features + ones row
W2_ROWS = [128, 128, 128, 42]  # w2*3 = 41 weight rows + b2 row
# gate order in the stacked [425, 600] gate weight: i_f g_f o_f i_b g_b o_b
GATE_FUNCS = [AF.Sigmoid, AF.Tanh, AF.Sigmoid] * 2
PG_BUFS = 4
N_WARM_MM = int(os.environ.get("KV_WARM", "4"))

# fp8 gate operand pack: [128, 4, PK8W] = per k-subtile (vecT | gate w6),
# zero-padded to 128 K-rows. Gates run as DoubleRow (2 k-subtiles per
# matmul) at 2x PE rate; the sigmoid/tanh nonlinearities and the small h
# share of cat attenuate the fp8 quantization error far below tolerance.
# DoubleRow requires lhsT inner free = 128, so each gate's 100-wide weight
# block is zero-padded to 128 (PSUM rows 100:128 unused).
PK8W = 384 + 6 * 128  # 1152: vecT cols 0:384, padded w6 cols 384:1152

# ---- packed bf16 constant layout: [128, NPK] ----
_SEGS = []  # name -> (rows, col_off, width)


def _seg(name, rows, width):
    off = _SEGS[-1][2] + _SEGS[-1][3] if _SEGS else 0
    _SEGS.append((name, rows, off, width))


for _k in range(4):
    _seg(f"vt{_k}", VT_ROWS[_k], 384)
for _g, _pre in enumerate(("w2i", "w2j")):
    for _k in range(4):
        _seg(f"{_pre}{_k}", W2_ROWS[_k], NREL)
    for _a in range(2):
        _seg(f"{_pre}h{_a}", 100, NREL)
SEG = {s[0]: s for s in _SEGS}
NPK = _SEGS[-1][2] + _SEGS[-1][3]
PK_CUTS = [SEG["vt3"][2] + SEG["vt3"][3], NPK]


def _build_program():
    nc = bacc.Bacc()

    pk_d = nc.dram_tensor("pk", [128, NPK], BF16, kind="ExternalInput")
    pk8_d = nc.dram_tensor("pk8", [128, 4, PK8W], FP8, kind="ExternalInput")
    bias_d = nc.dram_tensor("bias", [128, 6], F32, kind="ExternalInput")
    out_d = nc.dram_tensor("out", [128, NFLAT], BF16, kind="ExternalOutput")

    with TileContext(nc) as tc:
        with (
            tc.tile_pool(name="const", bufs=1) as cp,
            tc.tile_pool(name="work", bufs=3) as wp,
            tc.tile_pool(name="outp", bufs=5) as op_,
        ):
            # -------- early on-chip init (no DMA deps) --------
            wz = cp.tile([32, N_CHUNK], BF16, tag="wz")
            nc.gpsimd.memset(wz, 0.0)
            # lhsT of the pairwise matmul: rows 0:42 = g'_iT, row 42 = 1.0.
            # DVE partition base must be 32-aligned, so memset 32:43 and let
            # the later g' copy overwrite rows 32:42.
            el = cp.tile([NREL + 1, 128], BF16, tag="el")
            nc.vector.memset(el[32 : NREL + 1, :], 1.0)

            # -------- input DMAs (packed chunks + bias) --------
            # The first DoubleRow matmul needs k-subtiles 0 AND 1: one per
            # HWDGE queue so they land in parallel. On scalar, only the
            # subtile-1 issue goes ahead of the warmup activations — the
            # ACT table loads (~2.6us) must finish before the gate ACT
            # chain, so they run next, and the bulk DMAs (bias/vt/w2,
            # needed later) issue after them. k-subtile 3 only has 41
            # valid K-rows: memset the pad rows and ship just 0:42.
            pk8 = cp.tile([128, 4, PK8W], FP8, tag="pk8")
            for pb in (32, 64, 96):  # non-zero-base memsets max 32 partitions
                nc.vector.memset(pk8[pb : pb + 32, 3:4, :], 0.0)
            nc.sync.dma_start(out=pk8[:, 0:1, :], in_=pk8_d[:, 0:1, :])
            nc.scalar.dma_start(out=pk8[:, 1:2, :], in_=pk8_d[:, 1:2, :])
            nc.sync.dma_start(out=pk8[:, 2:3, :], in_=pk8_d[:, 2:3, :])
            nc.sync.dma_start(out=pk8[0:42, 3:4, :], in_=pk8_d[0:42, 3:4, :])
            bias = cp.tile([128, 6], F32, tag="bias")
            nc.scalar.dma_start(out=bias, in_=bias_d[:, :])
            pk = cp.tile([128, NPK], BF16, tag="pk")
            nc.scalar.dma_start(
                out=pk[:, 0 : PK_CUTS[0]], in_=pk_d[:, 0 : PK_CUTS[0]]
            )
            nc.scalar.dma_start(
                out=pk[:, PK_CUTS[0] : NPK], in_=pk_d[:, PK_CUTS[0] : NPK]
            )

            # warmup activations absorb the ACT table-set loads (after the
            # input issues: vt must land early or the scheduler stalls the
            # PE on vt-dependent matmuls it hoists ahead of the gate kk2s)
            warm2 = cp.tile([1, 8], F32, tag="warm2")
            nc.scalar.activation(out=warm2, in_=wz[0:1, 0:8], func=AF.Sigmoid)
            nc.scalar.activation(out=warm2, in_=wz[0:1, 0:8], func=AF.Tanh)
            # periodic identity block [42, IC_PER] generated on-chip in the
            # prologue shadow: memset 1.0, then keep only cols where
            # (col % 42) == partition via an affine iota m - r == 0 over a
            # [42, IC_PER//42, 42] view.
            ic = cp.tile([NREL, IC_PER], BF16, tag="ic")
            nc.gpsimd.memset(ic, 1.0)
            ic3d = bass.AP(
                tensor=ic.tensor,
                offset=ic.offset,
                ap=[ic.ap[0], [NREL, IC_PER // NREL], [1, NREL]],
            )
            nc.gpsimd.affine_select(
                out=ic3d,
                in_=ic3d,
                pattern=[[0, IC_PER // NREL], [1, NREL]],
                compare_op=mybir.AluOpType.is_equal,
                fill=0.0,
                base=0,
                channel_multiplier=-1,
            )

            def seg(name):
                _, rows, off, width = SEG[name]
                return pk[0:rows, off : off + width]

            vt = [seg(f"vt{k}") for k in range(4)]
            w2i = [seg(f"w2i{k}") for k in range(4)]
            w2ih = [seg(f"w2ih{a}") for a in range(2)]
            w2j = [seg(f"w2j{k}") for k in range(4)]
            w2jh = [seg(f"w2jh{a}") for a in range(2)]

            # pairwise rhs: rows 0:42 = periodic identity, row 42 = g'_j
            # flat. Replication from the on-chip ic block, split across the
            # scalar and sync HWDGE queues so the halves run in parallel.
            # (issued after the input chunks on both queues so the
            # replication transfers don't contend with the input landing)
            rr = cp.tile([NREL + 1, NFLAT], BF16, tag="rr")
            ic_rep = bass.AP(
                tensor=ic.tensor,
                offset=ic.offset,
                ap=[ic.ap[0], [0, NFLAT // IC_PER // 2], ic.ap[1]],
            )
            nc.sync.dma_start(out=rr[0:NREL, 0 : NFLAT // 2], in_=ic_rep)
            nc.scalar.dma_start(out=rr[0:NREL, NFLAT // 2 : NFLAT], in_=ic_rep)

            with tc.tile_pool(name="psum_pre", bufs=1, space="PSUM") as pp:
                # PE clock warmup during the input-DMA wait: a few matmuls
                # on the zeroed tile raise the PE activity monitor's clock
                # before the first real gate matmul.
                if N_WARM_MM:
                    pwarm = pp.tile([128, N_CHUNK], F32, tag="pg", bufs=PG_BUFS)
                    for _ in range(N_WARM_MM):
                        nc.tensor.matmul(
                            pwarm, lhsT=wz[:, 0:128], rhs=wz, start=True, stop=True
                        )

                # -------- LSTM gates (both dirs, f-gate skipped) --------
                # Per-direction ordering: i, g (then c = sig(i)*tanh(g) and
                # tanh(c) start immediately), then o, then h — shortens the
                # serial ACT chain to each direction's h.
                # gate pipeline runs full 128 partitions (pad rows compute
                # sigmoid(0)/tanh(0) — finite and discarded downstream)
                def gate_mm(m):
                    pg = pp.tile([128, T], F32, tag="pg", bufs=PG_BUFS, name=f"pg{m}")
                    for kk in (0, 2):
                        nc.tensor.matmul(
                            pg,
                            lhsT=pk8[:, kk : kk + 2, 384 + m * 128 : 512 + m * 128],
                            rhs=pk8[:, kk : kk + 2, 0:384],
                            start=(kk == 0),
                            stop=(kk == 2),
                            perf_mode=DOUBLE_ROW,
                        )
                    return pg

                def gate_act(pg, m, a_, sl):
                    nc.scalar.activation(
                        out=a_[:, sl],
                        in_=pg[:, sl],
                        func=GATE_FUNCS[m],
                        bias=bias[:, m : m + 1],
                        scale=1.0,
                    )

                FULL = slice(0, T)
                aa = [
                    wp.tile([128, T], BF16, tag=f"act{m}", name=f"act{m}")
                    for m in range(6)
                ]
                hh = []
                # d0: full-width (its ACTs precede d1's in the queue anyway)
                pg0 = gate_mm(0)
                gate_act(pg0, 0, aa[0], FULL)
                pg1 = gate_mm(1)
                gate_act(pg1, 1, aa[1], FULL)
                c0 = wp.tile([128, T], BF16, tag="c0")
                nc.vector.tensor_mul(c0, aa[0], aa[1])
                tc0 = wp.tile([128, T], BF16, tag="tc0")
                nc.scalar.activation(out=tc0, in_=c0, func=AF.Tanh)
                pg2 = gate_mm(2)
                gate_act(pg2, 2, aa[2], FULL)
                h0 = cp.tile([128, T], BF16, tag="h0")
                nc.vector.tensor_mul(h0, aa[2], tc0)
                hh.append(h0)
                # d1: j-columns first — h1[:, 128:384] unblocks the png
                # h-closers and the critical flatten DMA ~0.5us earlier;
                # the i-column parts run in otherwise-idle ACT time.
                pg3 = gate_mm(3)
                pg4 = gate_mm(4)
                pg5 = gate_mm(5)
                c1 = wp.tile([128, T], BF16, tag="c1")
                tc1 = wp.tile([128, T], BF16, tag="tc1")
                h1 = cp.tile([128, T], BF16, tag="h1")
                for sl in (slice(128, T), slice(0, 128)):
                    gate_act(pg3, 3, aa[3], sl)
                    gate_act(pg4, 4, aa[4], sl)
                    nc.vector.tensor_mul(c1[:, sl], aa[3][:, sl], aa[4][:, sl])
                    nc.scalar.activation(out=tc1[:, sl], in_=c1[:, sl], func=AF.Tanh)
                    gate_act(pg5, 5, aa[5], sl)
                    nc.vector.tensor_mul(h1[:, sl], aa[5][:, sl], tc1[:, sl])
                hh.append(h1)

                # -------- fused mlp: g'_j natural, then g'_iT ------------
                # Emission order = PE order: both c-blocks' vec matmuls
                # (no h dependency) run under the gate ACT chain, the
                # h-closers + flatten DMAs go out as early as possible, and
                # the i-block (pgi/el, ~2us of PE work the first pairwise
                # chunk also needs) fills the flatten completion latency.
                pngs = []
                for c in range(2):
                    cs = slice(128 + c * 128, 256 + c * 128)
                    png = pp.tile([128, NREL], F32, tag="pf", bufs=3, name=f"png{c}")
                    for k in range(4):
                        nc.tensor.matmul(
                            png,
                            lhsT=vt[k][0 : W2_ROWS[k], cs],
                            rhs=w2j[k],
                            start=(k == 0),
                            stop=False,
                        )
                    pngs.append(png)
                for c in range(2):
                    cs = slice(128 + c * 128, 256 + c * 128)
                    for a in range(2):
                        nc.tensor.matmul(
                            pngs[c],
                            lhsT=hh[a][0:100, cs],
                            rhs=w2jh[a],
                            start=False,
                            stop=(a == 1),
                        )
                    tj = wp.tile([128, NREL], BF16, tag=f"tj{c}")
                    nc.vector.tensor_copy(tj, pngs[c])
                    base = c * 128 * NREL
                    if c == 0:
                        # critical flatten: split across both HWDGE queues
                        # (the cost is ~per-packet, 128 x 84B), halving the
                        # completion latency before the first pairwise chunk
                        nc.sync.dma_start(
                            out=rr[NREL : NREL + 1, base : base + 64 * NREL],
                            in_=tj[0:64, :],
                        )
                        nc.scalar.dma_start(
                            out=rr[NREL : NREL + 1, base + 64 * NREL : base + 128 * NREL],
                            in_=tj[64:128, :],
                        )
                    else:
                        # c1 is only needed ~7us into the pairwise stream;
                        # its latency hides, so keep it off the HWDGE queues
                        nc.gpsimd.dma_start(
                            out=rr[NREL : NREL + 1, base : base + 128 * NREL],
                            in_=tj,
                        )

                # i-block: g'_iT[r, t] = sum_k W2[k, r] * cat[k, t]; the b2
                # bias arrives via vt3's ones row x w2i3's b2 row.
                pgi = pp.tile([NREL, 128], F32, tag="pf", bufs=3, name="pgi")
                for k in range(4):
                    nc.tensor.matmul(
                        pgi,
                        lhsT=w2i[k],
                        rhs=vt[k][0 : W2_ROWS[k], 0:128],
                        start=(k == 0),
                        stop=False,
                    )
                for a in range(2):
                    nc.tensor.matmul(
                        pgi,
                        lhsT=w2ih[a],
                        rhs=hh[a][0:100, 0:128],
                        start=False,
                        stop=(a == 1),
                    )
                nc.vector.tensor_copy(el[0:NREL, :], pgi)

            # -------- pairwise: tanh(g'_i + g'_j) --------
            # Group sizes: small first group lets the (bottleneck) ACT
            # tanh stream start early; small last group keeps the tail
            # DMA short.
            grp_plan = (1, 2, 3, 3, 3, 3, 3, 2, 1)
            with tc.tile_pool(name="psum_pair", bufs=2, space="PSUM") as pq:
                c = 0
                for nch in grp_plan:
                    ppair = pq.tile([128, GRP * N_CHUNK], F32, tag="ppair")
                    base = c * N_CHUNK
                    for q in range(nch):
                        nc.tensor.matmul(
                            ppair[:, q * N_CHUNK : (q + 1) * N_CHUNK],
                            lhsT=el,
                            rhs=rr[:, (c + q) * N_CHUNK : (c + q + 1) * N_CHUNK],
                            start=True,
                            stop=True,
                        )
                    ot = op_.tile([128, GRP * N_CHUNK], BF16, tag="ot")
                    nc.scalar.activation(
                        out=ot[:, 0 : nch * N_CHUNK],
                        in_=ppair[:, 0 : nch * N_CHUNK],
                        func=AF.Tanh,
                    )
                    # output groups on sync (a scalar issue mid-stream
                    # would steal ACT time from the tanh bottleneck); the
                    # last two groups split across both queues — ACT is
                    # finishing by then and the parallel drain is shorter
                    if c >= N_CHUNKS - 3:
                        half = nch * N_CHUNK // 2
                        nc.sync.dma_start(
                            out=out_d[:, base : base + half], in_=ot[:, 0:half]
                        )
                        nc.scalar.dma_start(
                            out=out_d[:, base + half : base + nch * N_CHUNK],
                            in_=ot[:, half : nch * N_CHUNK],
                        )
                    else:
                        nc.sync.dma_start(
                            out=out_d[:, base : base + nch * N_CHUNK],
                            in_=ot[:, 0 : nch * N_CHUNK],
                        )
                    c += nch

    nc.finalize()
    return nc


def _host_prepare(inputs):
    """Gather embeddings + lay out weights; returns per-core in_maps."""
    bf = ml_dtypes.bfloat16
    wi = np.asarray(inputs["word_idx"]).astype(np.int64)
    pi = np.asarray(inputs["pos_idx"]).astype(np.int64)
    ei = np.asarray(inputs["ext_idx"]).astype(np.int64)
    we = np.asarray(inputs["word_emb"], np.float32)
    pe = np.asarray(inputs["pos_emb"], np.float32)
    xe = np.asarray(inputs["ext_emb"], np.float32)
    vec = np.concatenate([we[wi], pe[pi], xe[ei]], axis=-1)  # [512, 425] f32

    w_ih_f = np.asarray(inputs["w_ih_f"], np.float32)
    w_ih_b = np.asarray(inputs["w_ih_b"], np.float32)
    b_f = np.asarray(inputs["b_f"], np.float32)
    b_b = np.asarray(inputs["b_b"], np.float32)
    w_mlp_in = np.asarray(inputs["w_mlp_in"], np.float32)
    b_mlp_in = np.asarray(inputs["b_mlp_in"], np.float32)
    w_mlp_out = np.asarray(inputs["w_mlp_out"], np.float32)
    b_mlp_out = np.asarray(inputs["b_mlp_out"], np.float32)

    # stacked gate weights [425, 600]: i_f g_f o_f i_b g_b o_b (f unused)
    w6 = np.concatenate(
        [
            w_ih_f[0:100],
            w_ih_f[200:300],
            w_ih_f[300:400],
            w_ih_b[0:100],
            w_ih_b[200:300],
            w_ih_b[300:400],
        ],
        axis=0,
    ).T  # [425, 600]

    bias = np.zeros((128, 6), np.float32)
    for m, sl in enumerate(
        [b_f[0:100], b_f[200:300], b_f[300:400], b_b[0:100], b_b[200:300], b_b[300:400]]
    ):
        bias[0:100, m] = sl

    # fused mlp: g' = cat @ W2.T + b2 (per-side), W2 = Wout @ Wmlp_in
    w2 = w_mlp_out @ w_mlp_in  # [42, 625]
    b2 = w_mlp_out @ b_mlp_in + 0.5 * b_mlp_out  # [42]

    def w2_parts(hv):
        if hv:  # cat = [h, vec]
            w2h = w2[:, 0:200].T  # [200, 42] rows = h features
            w2v = w2[:, 200:625].T  # [425, 42] rows = vec features
        else:  # cat = [vec, h]
            w2h = w2[:, 425:625].T
            w2v = w2[:, 0:425].T
        return w2h, w2v

    def fill(pk, name, arr):
        _, rows, off, width = SEG[name]
        assert arr.shape == (rows, width), (name, arr.shape, rows, width)
        pk[0:rows, off : off + width] = arr

    fp8 = ml_dtypes.float8_e4m3
    in_maps = []
    for core in range(8):
        ib, jh = core // 2, core % 2
        toks = np.concatenate(
            [np.arange(ib * 128, (ib + 1) * 128), np.arange(jh * 256, (jh + 1) * 256)]
        )
        vect = vec[toks].T  # [425, 384]

        # fp8 DoubleRow gate pack: [128, 4, 1152] = (vecT | w6pad) per
        # k-subtile; each gate's weights padded 100 -> 128 columns
        pk8 = np.zeros((128, 4, PK8W), np.float32)
        for k, (a, b) in enumerate(KS):
            pk8[0 : b - a, k, 0:384] = vect[a:b]
            for m in range(6):
                pk8[0 : b - a, k, 384 + m * 128 : 484 + m * 128] = w6[
                    a:b, m * 100 : (m + 1) * 100
                ]

        pk = np.zeros((128, NPK), np.float32)
        for k, (a, b) in enumerate(KS):
            vblk = vect[a:b]
            if k == 3:  # append the ones row for the b2 rank-1 bias fold
                vblk = np.concatenate([vblk, np.ones((1, T), np.float32)], axis=0)
            fill(pk, f"vt{k}", vblk)
        for pre, hv in (("w2i", ib < 2), ("w2j", jh == 0)):
            w2h, w2v = w2_parts(hv)
            for k, (a, b) in enumerate(KS):
                blk = w2v[a:b]
                if k == 3:
                    blk = np.concatenate([blk, b2[None, :]], axis=0)
                fill(pk, f"{pre}{k}", blk)
            for a in range(2):
                fill(pk, f"{pre}h{a}", w2h[a * 100 : (a + 1) * 100])
        in_maps.append(dict(pk=pk.astype(bf), pk8=pk8.astype(fp8), bias=bias))
    return in_maps


_CACHED_NC = None


def kernel(**inputs):
    global _CACHED_NC
    in_maps = _host_prepare(inputs)
    if _CACHED_NC is None:
        _CACHED_NC = _build_program()
    res = run_bass_kernel_spmd(_CACHED_NC, in_maps, list(range(8)))
    full = np.empty((SEQ, SEQ, NREL), np.float32)
    for core in range(8):
        ib, jh = core // 2, core % 2
        blk = np.asarray(res.results[core]["out"], dtype=np.float32).reshape(
            128, 256, NREL
        )
        full[ib * 128 : (ib + 1) * 128, jh * 256 : (jh + 1) * 256, :] = blk
    return full


if __name__ == "__main__":
    rng = np.random.default_rng(0)
    demo = dict(
        word_idx=rng.integers(0, 50000, 512),
        pos_idx=rng.integers(0, 48, 512),
        ext_idx=rng.integers(0, 100000, 512),
        word_emb=rng.standard_normal((50000, 100), np.float32) * 0.05,
        pos_emb=rng.standard_normal((48, 25), np.float32) * 0.05,
        ext_emb=rng.standard_normal((100000, 300), np.float32) * 0.05,
        w_ih_f=rng.standard_normal((400, 425), np.float32) * 0.05,
        b_f=rng.standard_normal(400).astype(np.float32) * 0.05,
        w_ih_b=rng.standard_normal((400, 425), np.float32) * 0.05,
        b_b=rng.standard_normal(400).astype(np.float32) * 0.05,
        w_mlp_in=rng.standard_normal((400, 625), np.float32) * 0.05,
        b_mlp_in=rng.standard_normal(400).astype(np.float32) * 0.05,
        w_mlp_out=rng.standard_normal((42, 400), np.float32) * 0.05,
        b_mlp_out=rng.standard_normal(42).astype(np.float32) * 0.05,
    )
    out = kernel(**demo)
    print("out", out.shape, out.dtype, float(np.abs(out).max()))
